# revision 6
# baseline (speedup 1.0000x reference)
"""v2: per-row staggered pipeline + PSUM-scalar iterations.

Row r's tail (transpose, exp, 49 iters, compaction, output) depends only on
row r's V columns, so Tile overlaps row 0's tail with row 1's einsum DMA.
Per-row tiles live on partitions 0..63.
"""

import numpy as np

B, N, D = 16, 8192, 512
NCORES = 8
RPC = B // NCORES
NUM_TOKENS = 1024
KEFF = 1152.0
SHIFT = 45.0
N_TAIL_ITERS = 49

_cache = {}


def _build_module():
    import concourse.bacc as bacc
    import concourse.mybir as mybir
    import concourse.tile as tile

    f32 = mybir.dt.float32
    i32 = mybir.dt.int32
    u32 = mybir.dt.uint32
    Alu = mybir.AluOpType
    Act = mybir.ActivationFunctionType

    nc = bacc.Bacc("TRN2", target_bir_lowering=False)

    x = nc.dram_tensor("x", [RPC, N, D], f32, kind="ExternalInput")
    rtb = nc.dram_tensor("rtb", [128, D], f32, kind="ExternalInput")
    jp1 = nc.dram_tensor("jp1", [64, 128], f32, kind="ExternalInput")
    blk = nc.dram_tensor("blk", [64, 64], f32, kind="ExternalInput")
    ident = nc.dram_tensor("ident", [128, 128], f32, kind="ExternalInput")
    out_idx = nc.dram_tensor("out_idx", [RPC, NUM_TOKENS], i32, kind="ExternalOutput")

    a1 = float(np.float32(np.log(np.float32(KEFF)) - np.float32(np.log(np.float32(N)))))

    with tile.TileContext(nc) as tc:
        with (
            tc.tile_pool(name="const", bufs=1) as constp,
            tc.tile_pool(name="xbig", bufs=2) as xp,
            tc.tile_pool(name="work", bufs=1) as wp,
            tc.tile_pool(name="it", bufs=2) as itp,
            tc.tile_pool(name="psum", bufs=2, space="PSUM") as pp,
            tc.tile_pool(name="psumc", bufs=2, space="PSUM") as pcp,
        ):
            rtb_sb = constp.tile([128, D], f32)
            nc.sync.dma_start(rtb_sb[:, :], rtb[:, :])
            jp1_sb = constp.tile([64, 128], f32)
            nc.sync.dma_start(jp1_sb[:, :], jp1[:, :])
            blk_sb = constp.tile([64, 64], f32)
            nc.sync.dma_start(blk_sb[:, :], blk[:, :])
            id_sb = constp.tile([128, 128], f32)
            nc.sync.dma_start(id_sb[:, :], ident[:, :])
            ones_row = constp.tile([1, 64], f32)
            nc.vector.memset(ones_row[:, :], 1.0)
            nega1 = constp.tile([64, 1], f32)
            nc.vector.memset(nega1[:, :], -a1)

            x5 = x.rearrange("r (a m c) d -> r a m (c d)", a=4, m=128, c=16)
            junk = wp.tile([128, D], f32, tag="junk")

            for r in range(RPC):
                V = wp.tile([128, 64], f32, tag=f"V{r}")
                # ---- einsum for this row ----
                for a in range(4):
                    xt = xp.tile([128, 16 * D], f32, tag="xt")
                    nc.sync.dma_start(xt[:, :], x5[r, a])
                    xt3 = xt[:, :].rearrange("m (c d) -> m c d", c=16)
                    for c in range(16):
                        col = a * 16 + c
                        nc.vector.scalar_tensor_tensor(
                            out=junk[:, :],
                            in0=xt3[:, c],
                            scalar=1.0,
                            in1=rtb_sb[:, :],
                            op0=Alu.mult,
                            op1=Alu.mult,
                            accum_out=V[:, col : col + 1],
                        )

                # ---- transpose -> W_r [64, 128] ----
                Wps = pp.tile([64, 128], f32, tag="wps")
                nc.tensor.transpose(Wps[:, :], V[:, :], id_sb[:, :])
                W = wp.tile([64, 128], f32, tag=f"W{r}")
                nc.vector.tensor_copy(W[:, :], Wps[:, :])

                # ---- row max, E, c1 ----
                pmax = wp.tile([64, 1], f32, tag=f"pmax{r}")
                nc.vector.reduce_max(pmax[:, :], W[:, :], axis=mybir.AxisListType.X)
                pmax_t = pp.tile([1, 64], f32, tag="pmaxt")
                nc.tensor.transpose(pmax_t[:, :], pmax[:, :], id_sb[:64, :64])
                pmax_ts = wp.tile([1, 64], f32, tag=f"pmaxts{r}")
                nc.vector.tensor_copy(pmax_ts[:, :], pmax_t[:, :])
                rmax = wp.tile([1, 1], f32, tag=f"rmax{r}")
                nc.vector.reduce_max(
                    rmax[:, :], pmax_ts[:, :], axis=mybir.AxisListType.X
                )
                Mb = pp.tile([64, 1], f32, tag="mb")
                nc.tensor.matmul(
                    Mb[:, :], lhsT=ones_row[:, :], rhs=rmax[:, :],
                    start=True, stop=True,
                )
                negb = wp.tile([64, 1], f32, tag=f"negb{r}")
                nc.scalar.activation(
                    negb[:, :], Mb[:, :], Act.Copy, bias=SHIFT, scale=-1.0
                )
                E = wp.tile([64, 128], f32, tag=f"E{r}")
                nc.scalar.activation(
                    E[:, :], W[:, :], Act.Exp, bias=negb[:, :], scale=1.0
                )

                # ---- c recursion (scalar read straight from PSUM) ----
                c_cur = pcp.tile([64, 1], f32, tag="c")
                nc.scalar.activation(
                    c_cur[:, :], negb[:, :], Act.Exp, bias=nega1[:, :], scale=1.0
                )
                U = wp.tile([64, 128], f32, tag="U")
                for t in range(N_TAIL_ITERS):
                    partial = itp.tile([64, 1], f32, tag="partial")
                    nc.vector.tensor_scalar(
                        out=U[:, :],
                        in0=E[:, :],
                        scalar1=c_cur[:, :],
                        scalar2=None,
                        op0=Alu.min,
                        op1=Alu.add,
                        accum_out=partial[:, :],
                    )
                    c_next = pcp.tile([64, 1], f32, tag="c")
                    nc.tensor.matmul(
                        c_next[:, :], lhsT=blk_sb[:, :], rhs=partial[:, :],
                        start=True, stop=True,
                    )
                    c_cur = c_next

                # ---- candidates + wrap + compact + emit ----
                cand = wp.tile([64, 128], f32, tag=f"cand{r}")
                nc.vector.scalar_tensor_tensor(
                    out=cand[:, :],
                    in0=E[:, :],
                    scalar=c_cur[:, :],
                    in1=jp1_sb[:, :],
                    op0=Alu.is_ge,
                    op1=Alu.mult,
                )
                nc.vector.tensor_scalar_add(cand[:, :], cand[:, :], -1.0)
                W3 = wp.tile([16, 512], f32, tag=f"W3_{r}")
                for a in range(4):
                    nc.sync.dma_start(
                        W3[:, a * 128 : (a + 1) * 128],
                        cand[a * 16 : (a + 1) * 16, :],
                    )
                gath = wp.tile([16, 128], f32, tag=f"gath_{r}")
                nfound = wp.tile([1, 1], u32, tag=f"nf_{r}")
                nc.gpsimd.sparse_gather(gath[:, :], W3[:, :], num_found=nfound[:, :])
                idx32 = wp.tile([16, 64], i32, tag=f"idx_{r}")
                nc.vector.tensor_copy(idx32[:, :], gath[:, :64])
                nc.sync.dma_start(
                    out_idx[r].rearrange("(f p) -> p f", p=16),
                    idx32[:, :],
                )

    nc.compile()
    return nc


def _consts():
    p = np.arange(64)
    a = p // 16
    c = p % 16
    jp1 = ((a * 2048 + c)[:, None] + np.arange(128)[None, :] * 16 + 1).astype(
        np.float32
    )
    blk = np.full((64, 64), np.float32(1.0) / np.float32(KEFF), np.float32)
    ident = np.eye(128, dtype=np.float32)
    return jp1, blk, ident


def kernel(x, routing_token, num_tokens):
    from concourse.bass_utils import run_bass_kernel_spmd

    x = np.ascontiguousarray(np.asarray(x, dtype=np.float32))
    routing_token = np.asarray(routing_token, dtype=np.float32)
    num_tokens = int(num_tokens)
    assert x.shape == (B, N, D) and num_tokens == NUM_TOKENS

    if "nc" not in _cache:
        _cache["nc"] = _build_module()
    nc = _cache["nc"]

    jp1, blk, ident = _consts()
    rtb = np.ascontiguousarray(
        np.broadcast_to(routing_token[0], (128, D)).astype(np.float32)
    )
    in_maps = [
        {
            "x": x[core * RPC : (core + 1) * RPC],
            "rtb": rtb,
            "jp1": jp1,
            "blk": blk,
            "ident": ident,
        }
        for core in range(NCORES)
    ]
    res = run_bass_kernel_spmd(nc, in_maps, core_ids=list(range(NCORES)))
    idx = np.concatenate([r["out_idx"] for r in res.results], axis=0)
    scores = np.ones((B, NUM_TOKENS), np.float32)
    return scores, idx.astype(np.int32)
